# revision 1
# baseline (speedup 1.0000x reference)
"""Trainium2 Bass kernel for the 2-layer grid-GCN + linear head.

Math: the GCN aggregation over the fixed 26x26 grid is a banded linear
operator on the node axis (halfwidth 26): per batch column

    h1 = relu(B1 @ xT + b1)      B1 = w1 * A
    h2 = relu(B2 @ h1 + b2)      B2 = w2 * A
    y  = relu(lin_w @ h2 + lin_b)

Tiling: three node tilings, each shifted by the band halfwidth, so every
conv output tile [128k-26, 128k+102) needs exactly one full input tile
plus the first 52 rows of the next one (2 matmuls, K<=128, one PSUM
accumulation group; no halo duplication anywhere):

    x  tiles [0,128) [76,204) [204,332) ... (tile 0 widened: no halo)
    h1 tiles start at [0, 102, 230, 358, 486, 614]
    h2 tiles start at [0, 128, 256, 384, 512, 640]

The head is folded into conv2: lin_w (signed) is multiplied into the
conv2 stationary rows, so psum_j = lw_j * v_j and
    lw_j * relu(v_j + b2) = s_j * relu(s_j*psum_j + |lw_j|*b2)
with s = sign(lw) applied as a per-partition activation scale (Act
engine), followed by a chained scalar_tensor_tensor on the vector
engine: acc = u * s + acc.  A ones-vector matmul reduces the 128
accumulator partitions to y.

Batch is sharded across the 8 NeuronCores (pure data parallel); x is
transposed and cast to bf16 on the host.  PSUM tiles are [*, 1024] f32
(two banks = two 512-column chunks) so each drain/chain op covers two
chunks.  conv1 and conv2 stages are interleaved in a staircase (conv2
stage k right after conv1 stage k+1) to keep all engines evenly loaded.
Per 512-column chunk the tensor engine runs 22 matmuls (10 conv1 + 11
conv2 + 1 head); conv1 drains split between vector and scalar engines.
"""

import sys

if "/opt/trn_rl_repo" not in sys.path:
    sys.path.insert(0, "/opt/trn_rl_repo")

import numpy as np
import ml_dtypes

N_CORES = 8
N = 676                        # nodes (26x26 grid)
B_TOTAL = 65536
COLS = B_TOTAL // N_CORES      # batch columns per core (8192)
CHUNK = 512                    # matmul moving free dim / PSUM bank
PAIR = 2 * CHUNK               # psum tile width (2 banks)
GROUP = 2048                   # x DMA column-group (4 chunks)
N_CHUNKS = COLS // CHUNK       # 16
N_GROUPS = COLS // GROUP       # 4
HALO = 52                      # 2 * band halfwidth

XLO = [0, 76, 204, 332, 460, 588]         # x tile starts
XHI = [128, 204, 332, 460, 588, 676]      # x tile ends (tile 0 widened)
HS = [0, 102, 230, 358, 486, 614, 676]    # h1 tiling
ZS = [0, 128, 256, 384, 512, 640, 676]    # h2 tiling
NT = 6

PX = [XHI[k] - XLO[k] for k in range(NT)]
PH = [HS[k + 1] - HS[k] for k in range(NT)]
PZ = [ZS[k + 1] - ZS[k] for k in range(NT)]

bf16 = ml_dtypes.bfloat16

TRACE = False            # test.py flips this to profile
LAST_RESULT = None       # BassKernelResults stash when TRACE

_PROGRAM_CACHE = {}


def _build_program(b1f, b2f, linbf):
    key = (b1f, b2f, linbf)
    if key in _PROGRAM_CACHE:
        return _PROGRAM_CACHE[key]

    import concourse.mybir as mybir
    import concourse.tile as tile
    import concourse.bass_isa as bass_isa
    from concourse import bacc

    nc = bacc.Bacc(None, target_bir_lowering=False)
    dt = mybir.dt
    relu = mybir.ActivationFunctionType.Relu
    Alu = mybir.AluOpType
    RADD = bass_isa.ReduceOp.add

    xt_d = nc.dram_tensor("xt", (N, COLS), dt.bfloat16, kind="ExternalInput")
    w1m_d = [nc.dram_tensor(f"w1m{k}", (PX[k], PH[k]), dt.bfloat16,
                            kind="ExternalInput") for k in range(NT)]
    # conv1 halo only for k=1..4 (tile 0 is widened to a full 128 window)
    w1h_d = {k: nc.dram_tensor(f"w1h{k}", (HALO, PH[k]), dt.bfloat16,
                               kind="ExternalInput") for k in range(1, NT - 1)}
    w2m_d = [nc.dram_tensor(f"w2m{k}", (PH[k], 128), dt.bfloat16,
                            kind="ExternalInput") for k in range(NT)]
    w2h_d = [nc.dram_tensor(f"w2h{k}", (HALO, 128), dt.bfloat16,
                            kind="ExternalInput") for k in range(NT - 1)]
    ones_d = nc.dram_tensor("ones", (128, 1), dt.bfloat16, kind="ExternalInput")
    sgn_d = nc.dram_tensor("sgn", (128, NT), dt.float32, kind="ExternalInput")
    wb2_d = nc.dram_tensor("wb2", (128, NT), dt.float32, kind="ExternalInput")
    y_d = nc.dram_tensor("y", (1, COLS), dt.float32, kind="ExternalOutput")

    with tile.TileContext(nc) as tc:
        with (
            tc.tile_pool(name="weights", bufs=1) as wpool,
            tc.tile_pool(name="xin", bufs=2) as xpool,
            tc.tile_pool(name="h1", bufs=2) as hpool,
            tc.tile_pool(name="acc", bufs=2) as apool,
            tc.tile_pool(name="yout", bufs=1) as ypool,
            tc.tile_pool(name="ps", bufs=3, space="PSUM") as pspool,
            tc.tile_pool(name="psl", bufs=2, space="PSUM") as pslpool,
        ):
            xt_t = [[None] * N_GROUPS for _ in range(NT)]

            dma_engines = [nc.sync, nc.scalar]

            def dma_x_half(k, g, h):
                """DMA half-tile h (1024 cols) of x tile k, group g."""
                if xt_t[k][g] is None:
                    xt_t[k][g] = [None, None]
                t = xpool.tile([PX[k], PAIR], dt.bfloat16,
                               tag=f"x{k}{h}", name=f"x{k}{h}_{g}")
                xt_t[k][g][h] = t
                # spread x DMAs over two hardware DGE queues
                eng = dma_engines[(k + h) % 2]
                c0 = g * GROUP + h * PAIR
                eng.dma_start(t[:], xt_d[XLO[k]:XHI[k], c0:c0 + PAIR])

            def dma_x_tile(k, g):
                dma_x_half(k, g, 0)
                dma_x_half(k, g, 1)

            # stage-0 weights and first x tiles first so compute starts ASAP
            w1m = [wpool.tile([PX[k], PH[k]], dt.bfloat16, tag=f"w1m{k}",
                              name=f"w1m{k}") for k in range(NT)]
            w1h = {k: wpool.tile([HALO, PH[k]], dt.bfloat16, tag=f"w1h{k}",
                                 name=f"w1h{k}") for k in range(1, NT - 1)}
            w2m = [wpool.tile([PH[k], 128], dt.bfloat16, tag=f"w2m{k}",
                              name=f"w2m{k}") for k in range(NT)]
            w2h = [wpool.tile([HALO, 128], dt.bfloat16, tag=f"w2h{k}",
                              name=f"w2h{k}") for k in range(NT - 1)]
            ones = wpool.tile([128, 1], dt.bfloat16, tag="ones")
            sgn = wpool.tile([128, NT], dt.float32, tag="sgn")

            # just-in-time startup order matching the staircase:
            # c1s0, c1s1, c2s0, c1s2, c2s1, c1s3, ...
            nc.sync.dma_start(w1m[0][:], w1m_d[0][:])
            nc.sync.dma_start(w1m[1][:], w1m_d[1][:])
            nc.sync.dma_start(w1h[1][:], w1h_d[1][:])
            dma_x_half(0, 0, 0)
            dma_x_half(1, 0, 0)
            dma_x_half(2, 0, 0)
            nc.sync.dma_start(w1m[2][:], w1m_d[2][:])
            nc.sync.dma_start(w1h[2][:], w1h_d[2][:])
            nc.sync.dma_start(w2m[0][:], w2m_d[0][:])
            nc.sync.dma_start(w2h[0][:], w2h_d[0][:])
            dma_x_half(3, 0, 0)
            dma_x_half(0, 0, 1)
            dma_x_half(1, 0, 1)
            nc.sync.dma_start(w1m[3][:], w1m_d[3][:])
            nc.sync.dma_start(w1h[3][:], w1h_d[3][:])
            nc.sync.dma_start(w2m[1][:], w2m_d[1][:])
            nc.sync.dma_start(w2h[1][:], w2h_d[1][:])
            dma_x_half(4, 0, 0)
            dma_x_half(2, 0, 1)
            dma_x_half(3, 0, 1)
            nc.sync.dma_start(w1m[4][:], w1m_d[4][:])
            nc.sync.dma_start(w1h[4][:], w1h_d[4][:])
            nc.sync.dma_start(w2m[2][:], w2m_d[2][:])
            nc.sync.dma_start(w2h[2][:], w2h_d[2][:])
            dma_x_half(5, 0, 0)
            dma_x_half(4, 0, 1)
            dma_x_half(5, 0, 1)
            nc.sync.dma_start(w1m[5][:], w1m_d[5][:])
            for k in range(3, NT):
                nc.sync.dma_start(w2m[k][:], w2m_d[k][:])
                if k < NT - 1:
                    nc.sync.dma_start(w2h[k][:], w2h_d[k][:])
            nc.sync.dma_start(ones[:], ones_d[:])
            nc.sync.dma_start(sgn[:], sgn_d[:])
            if b2f != 0.0:
                wb2 = wpool.tile([128, NT], dt.float32, tag="wb2")
                nc.sync.dma_start(wb2[:], wb2_d[:])

            for h in range(2):
                for k in range(NT):
                    dma_x_half(k, 1, h)

            y_sb = ypool.tile([1, COLS], dt.float32, tag="y")

            # accf[g][j]: final bf16 accumulator [128, PAIR] per chunk pair
            accf = [[None, None] for _ in range(N_GROUPS)]

            def head_phase(g):
                """Ones-matmul partition-reduce acc to y, then DMA out."""
                for j in range(2):
                    for c in range(2):
                        psl = pslpool.tile([1, CHUNK], dt.float32, tag="psl",
                                           name=f"psl{g}_{j}_{c}")
                        nc.tensor.matmul(
                            psl[:], ones[:],
                            accf[g][j][:, c * CHUNK:(c + 1) * CHUNK],
                            start=True, stop=True,
                        )
                        col = (g * 4 + j * 2 + c) * CHUNK
                        nc.scalar.activation(
                            y_sb[0:1, col:col + CHUNK], psl[:], relu,
                            bias=linbf,
                        )
                nc.sync.dma_start(
                    y_d[0:1, g * GROUP:(g + 1) * GROUP],
                    y_sb[0:1, g * GROUP:(g + 1) * GROUP],
                )

            for g in range(N_GROUPS):
                if 1 <= g < N_GROUPS - 1:
                    for h in range(2):
                        for k in range(NT):
                            dma_x_half(k, g + 1, h)

                h1_t = [None] * NT
                for k in range(NT):
                    h1_t[k] = hpool.tile([PH[k], GROUP], dt.bfloat16,
                                         tag=f"h1_{k}", name=f"h1_{k}_{g}")

                def conv1_stage(k):
                    # tile 0 has a full 128-row x window, tile 5 clips: no halo
                    has_halo = 1 <= k < NT - 1
                    ps1 = [None, None]
                    for j in range(2):   # chunk pairs (0,1) and (2,3)
                        ps1[j] = pspool.tile([PH[k], PAIR], dt.float32,
                                             tag="ps", name=f"ps1_{k}_{j}_{g}")
                    for j in range(2):
                        for c in range(2):
                            cc = slice(c * CHUNK, (c + 1) * CHUNK)
                            nc.tensor.matmul(
                                ps1[j][:, cc],
                                w1m[k][:], xt_t[k][g][j][:, cc],
                                start=True, stop=not has_halo,
                            )
                    if has_halo:
                        for j in range(2):
                            for c in range(2):
                                cc = slice(c * CHUNK, (c + 1) * CHUNK)
                                nc.tensor.matmul(
                                    ps1[j][:, cc],
                                    w1h[k][:], xt_t[k + 1][g][j][0:HALO, cc],
                                    start=False, stop=True,
                                )
                    for j in range(2):
                        # alternate drain engine so PSUM frees faster
                        dst = h1_t[k][:, j * PAIR:(j + 1) * PAIR]
                        if j == 0:
                            if b1f == 0.0:
                                nc.vector.tensor_scalar_max(
                                    dst, ps1[j][:], 0.0)
                            else:
                                nc.vector.tensor_scalar(
                                    dst, ps1[j][:], b1f, 0.0,
                                    Alu.add, Alu.max)
                        else:
                            nc.scalar.activation(
                                dst, ps1[j][:], relu, bias=b1f)

                acc_prev = [None, None]

                def conv2_stage(k):
                    last = k == NT - 1
                    ps2 = [None, None]
                    for j in range(2):
                        ps2[j] = pspool.tile([128, PAIR], dt.float32,
                                             tag="ps", name=f"ps2_{k}_{j}_{g}")
                    for j in range(2):
                        for c in range(2):
                            cs = slice((j * 2 + c) * CHUNK,
                                       (j * 2 + c + 1) * CHUNK)
                            nc.tensor.matmul(
                                ps2[j][:, c * CHUNK:(c + 1) * CHUNK],
                                w2m[k][:], h1_t[k][:, cs],
                                start=True, stop=last,
                            )
                    if not last:
                        for j in range(2):
                            for c in range(2):
                                cs = slice((j * 2 + c) * CHUNK,
                                           (j * 2 + c + 1) * CHUNK)
                                nc.tensor.matmul(
                                    ps2[j][:, c * CHUNK:(c + 1) * CHUNK],
                                    w2h[k][:], h1_t[k + 1][0:HALO, cs],
                                    start=False, stop=True,
                                )
                    # lw_j*relu(v+b2) = s_j*relu(s_j*psum + |lw_j|*b2)
                    # two independent per-pair chains keep latency short
                    bias = wb2[:, k:k + 1] if b2f != 0.0 else 0.0
                    for j in range(2):
                        u = apool.tile([128, PAIR], dt.bfloat16,
                                       tag=f"u{j}", name=f"u{j}_{k}_{g}")
                        nc.scalar.activation(
                            u[:], ps2[j][:], relu,
                            bias=bias, scale=sgn[:, k:k + 1],
                        )
                        a_new = apool.tile([128, PAIR], dt.bfloat16,
                                           tag=(f"accf{j}" if last
                                                else f"acc{j}_{k % 2}"),
                                           name=f"acc{j}_{k}_{g}")
                        if k == 0:
                            nc.vector.tensor_scalar_mul(
                                a_new[:], u[:], sgn[:, k:k + 1])
                        else:
                            nc.vector.scalar_tensor_tensor(
                                a_new[:], u[:], sgn[:, k:k + 1],
                                acc_prev[j][:], Alu.mult, Alu.add,
                            )
                        if last:
                            accf[g][j] = a_new
                        acc_prev[j] = a_new

                # staircase: conv2 stage k right after conv1 stage k+1
                conv1_stage(0)
                conv1_stage(1)
                if g >= 1:
                    head_phase(g - 1)
                for k in range(NT):
                    conv2_stage(k)
                    if k + 2 < NT:
                        conv1_stage(k + 2)

            head_phase(N_GROUPS - 1)

    nc.compile()
    _PROGRAM_CACHE[key] = nc
    return nc


def kernel(x, w1, b1, w2, b2, lin_w, lin_b, edge_src, edge_dst):
    global LAST_RESULT
    from concourse import bass_utils

    x = np.asarray(x)
    # Build the dense normalized aggregation operator from the edge lists.
    deg = np.zeros(N, np.float64)
    np.add.at(deg, np.asarray(edge_dst), 1.0)
    dinv = 1.0 / np.sqrt(deg)
    normv = dinv[np.asarray(edge_src)] * dinv[np.asarray(edge_dst)]
    A = np.zeros((N, N), np.float64)
    np.add.at(A, (np.asarray(edge_dst), np.asarray(edge_src)), normv)

    w1f = float(np.asarray(w1).reshape(-1)[0])
    w2f = float(np.asarray(w2).reshape(-1)[0])
    b1f = float(np.asarray(b1).reshape(-1)[0])
    b2f = float(np.asarray(b2).reshape(-1)[0])
    linbf = float(np.asarray(lin_b).reshape(-1)[0])
    lw = np.asarray(lin_w).reshape(-1).astype(np.float64)

    B1 = w1f * A
    B2s = (lw[:, None]) * (w2f * A)   # head weights folded into conv2 rows

    in_map = {}
    for k in range(NT):
        in_map[f"w1m{k}"] = np.ascontiguousarray(
            B1[HS[k]:HS[k + 1], XLO[k]:XHI[k]].T.astype(bf16))
        if 1 <= k < NT - 1:
            in_map[f"w1h{k}"] = np.ascontiguousarray(
                B1[HS[k]:HS[k + 1], XHI[k]:XHI[k] + HALO].T.astype(bf16))
        m2 = np.zeros((PH[k], 128), np.float64)
        m2[:, 0:PZ[k]] = B2s[ZS[k]:ZS[k + 1], HS[k]:HS[k + 1]].T
        in_map[f"w2m{k}"] = np.ascontiguousarray(m2.astype(bf16))
        if k < NT - 1:
            h2b = np.zeros((HALO, 128), np.float64)
            h2b[:, 0:PZ[k]] = B2s[ZS[k]:ZS[k + 1], HS[k + 1]:HS[k + 1] + HALO].T
            in_map[f"w2h{k}"] = np.ascontiguousarray(h2b.astype(bf16))
    in_map["ones"] = np.ones((128, 1), dtype=bf16)
    sgn = np.zeros((128, NT), np.float64)
    wb2 = np.zeros((128, NT), np.float32)
    for k in range(NT):
        sgn[0:PZ[k], k] = np.sign(lw[ZS[k]:ZS[k + 1]])
        wb2[0:PZ[k], k] = (np.abs(lw[ZS[k]:ZS[k + 1]]) * b2f).astype(np.float32)
    in_map["sgn"] = sgn.astype(np.float32)
    in_map["wb2"] = wb2

    nc = _build_program(b1f, b2f, linbf)

    # host-side: transpose, cast, shard along batch
    xt = np.ascontiguousarray(x.T).astype(bf16)        # [676, 65536]
    in_maps = []
    for c in range(N_CORES):
        m = dict(in_map)
        m["xt"] = np.ascontiguousarray(xt[:, c * COLS:(c + 1) * COLS])
        in_maps.append(m)

    res = bass_utils.run_bass_kernel_spmd(
        nc, in_maps, list(range(N_CORES)), trace=TRACE
    )
    if TRACE:
        LAST_RESULT = res
    out = np.concatenate([res.results[c]["y"].reshape(-1) for c in range(N_CORES)])
    return out.reshape(B_TOTAL, 1).astype(np.float32)

